# revision 4
# baseline (speedup 1.0000x reference)
"""GPT2 causal self-attention forward on 8 Trainium2 NeuronCores (Bass/Tile).

Contract: kernel(**inputs) takes the FULL inputs of reference.setup_inputs()
  hidden_states [4, 2048, 1024] f32, w_attn [1024, 3072] f32,
  b_attn [3072] f32, w_proj [1024, 1024] f32, b_proj [1024] f32
and returns the FULL output [4, 2048, 1024] f32.

Sharding (batch x head-half, no collectives): core c = 2*b + m handles batch b
and heads m*8..m*8+7.  Each core computes q/k/v for its 8 heads (columns of
w_attn), causal attention, and a partial output projection against its 512
rows of w_proj.  The host sums the two partial projections per batch and adds
b_proj (exact, since the partials split the contraction dimension).

Per-core device pipeline (all matmuls bf16 with f32 PSUM accumulation):
  hidT [D,S] (host-pre-transposed, bf16) -> qT/kT [c,S] and V [S,c] tiles;
  scoresT[k,q] for both heads of a pair land in one 2-bank PSUM tile so a
  single scalar-engine Exp covers them; causality via column slicing on
  diagonal tiles + a triangular 128x128 band multiply; PV with a
  ones-augmented V so PSUM row 64 carries the softmax denominator.
  Normalization is deferred: unnormalized PV numerators and denominator rows
  are copied out immediately (freeing PSUM), then once per q block a single
  batched [8,512] reciprocal + gpsimd partition broadcasts + one [128,512]
  multiply per head-pair apply 1/z.  QKV / projection matmul groups are
  interleaved between attention steps so the tensor engine stays busy while
  the scalar engine streams Exp.
"""

import contextlib
import sys

for _p in ("/opt/trn_rl_repo", "/root/.axon_site/_ro/trn_rl_repo"):
    if _p not in sys.path:
        sys.path.append(_p)

import numpy as np
import ml_dtypes

import concourse.mybir as mybir
import concourse.tile as tile
from concourse import bacc
from concourse.bass_utils import run_bass_kernel_spmd

FP32 = mybir.dt.float32
BF16 = mybir.dt.bfloat16
AF = mybir.ActivationFunctionType
OP = mybir.AluOpType

S, D, H, HD = 2048, 1024, 8, 64   # per-core: sequence, model dim, heads, head dim
P = 128
C = H * HD                         # 512 channels per core
KD = D // P                        # 8 contraction tiles over D
RC, QB = 512, 512                  # row chunk / query block
NRT, NRC, NQB = S // P, S // RC, S // QB
NCT = C // P                       # 4 channel tiles (2 heads each)
NPJ = D // 512                     # 2 projection column chunks

_NC_CACHE = {}


def _build_kernel(iters=1):
    nc = bacc.Bacc("TRN2", target_bir_lowering=False, debug=False, num_devices=8)

    hidT_d = nc.dram_tensor("hidT", [D, S], BF16, kind="ExternalInput")
    wq_d = nc.dram_tensor("wq", [D, C], BF16, kind="ExternalInput")
    wk_d = nc.dram_tensor("wk", [D, C], BF16, kind="ExternalInput")
    wv_d = nc.dram_tensor("wv", [D, C], BF16, kind="ExternalInput")
    bq_d = nc.dram_tensor("bq", [C], FP32, kind="ExternalInput")
    bk_d = nc.dram_tensor("bk", [C], FP32, kind="ExternalInput")
    bv_d = nc.dram_tensor("bv", [C], FP32, kind="ExternalInput")
    wp_d = nc.dram_tensor("wp", [C, D], BF16, kind="ExternalInput")
    out_d = nc.dram_tensor("out", [S, D], FP32, kind="ExternalOutput")

    with tile.TileContext(nc) as tc:
        with (
            tc.tile_pool(name="persist", bufs=1) as pp,
            tc.tile_pool(name="pt_pool", bufs=6) as ptp,
            tc.tile_pool(name="work", bufs=2) as wkp,
            tc.tile_pool(name="bzp", bufs=2) as bzp,
            tc.tile_pool(name="outp", bufs=3) as obp,
            tc.tile_pool(name="mm", bufs=2, space="PSUM") as mmp,
            tc.tile_pool(name="sc", bufs=2, space="PSUM") as scp,
            tc.tile_pool(name="pv", bufs=2, space="PSUM") as pvp,
        ):
          with (tc.For_i(0, iters) if iters > 1 else contextlib.nullcontext()):
            # ---- persistent tiles ----
            hidT = pp.tile([P, KD, S], BF16, tag="hidT", name="hidT_sb")
            wq = pp.tile([P, KD, C], BF16, tag="wq", name="wq_sb")
            wk = pp.tile([P, KD, C], BF16, tag="wk", name="wk_sb")
            wv = pp.tile([P, KD, C], BF16, tag="wv", name="wv_sb")
            wp = pp.tile([P, NCT, D], BF16, tag="wp", name="wp_sb")
            bq = pp.tile([P, NCT], FP32, tag="bq", name="bq_sb")
            bk = pp.tile([P, NCT], FP32, tag="bk", name="bk_sb")
            bv1 = pp.tile([1, C], FP32, tag="bv1", name="bv1_sb")
            bvb = pp.tile([P, H, HD], FP32, tag="bvb", name="bvb_sb")
            maskf = pp.tile([P, P], FP32, tag="maskf", name="maskf_sb")
            maskb = pp.tile([P, P], BF16, tag="maskb", name="maskb_sb")
            qT = [pp.tile([P, S], BF16, tag=f"qT{ct}", name=f"qT{ct}_sb")
                  for ct in range(NCT)]
            kT = [pp.tile([P, S], BF16, tag=f"kT{ct}", name=f"kT{ct}_sb")
                  for ct in range(NCT)]
            vaug = [pp.tile([P, H, HD + 1], BF16, tag=f"va{rt}", name=f"va{rt}_sb")
                    for rt in range(NRT)]
            aT = [pp.tile([P, S], BF16, tag=f"aT{ct}", name=f"aT{ct}_sb")
                  for ct in range(NCT)]
            zq = [pp.tile([H, QB], FP32, tag=f"zq{qb}", name=f"zq{qb}_sb")
                  for qb in range(NQB)]
            zinv = [pp.tile([H, QB], FP32, tag=f"zi{qb}", name=f"zi{qb}_sb")
                    for qb in range(NQB)]

            # ---- input DMA ----
            for j in range(KD):
                nc.sync.dma_start(hidT[:, j, :], hidT_d[j * P:(j + 1) * P, :])
                nc.sync.dma_start(wq[:, j, :], wq_d[j * P:(j + 1) * P, :])
                nc.sync.dma_start(wk[:, j, :], wk_d[j * P:(j + 1) * P, :])
                nc.sync.dma_start(wv[:, j, :], wv_d[j * P:(j + 1) * P, :])
            for ct in range(NCT):
                nc.sync.dma_start(wp[:, ct, :], wp_d[ct * P:(ct + 1) * P, :])
            nc.sync.dma_start(bq[:], bq_d.rearrange("(ct p) -> p ct", p=P))
            nc.sync.dma_start(bk[:], bk_d.rearrange("(ct p) -> p ct", p=P))
            nc.sync.dma_start(bv1[:], bv_d[None, :])

            # broadcast bv across partitions: bvb[p, h, dd] = bv[64h+dd]
            nc.gpsimd.partition_broadcast(
                bvb.rearrange("p h d -> p (h d)"), bv1[:], channels=P)

            # triangular mask band: maskb[kk, u] = 1 if u >= kk else 0
            nc.vector.memset(maskf[:], 1.0)
            nc.gpsimd.affine_select(
                out=maskf[:], in_=maskf[:], compare_op=OP.is_ge, fill=0.0,
                base=0, channel_multiplier=-1, pattern=[[1, P]])
            nc.vector.tensor_copy(maskb[:], maskf[:])

            # ---- PE work generators ----
            def qkv_groups(rc):
                """QKV matmul groups for sequence chunk rc (list of thunks)."""
                gs = []
                cols = slice(rc * RC, (rc + 1) * RC)

                def qk_group(ct, wt, bt, dst):
                    ccols = slice(ct * P, (ct + 1) * P)
                    ps = mmp.tile([P, RC], FP32, tag="mm", name="qk_ps")
                    for j in range(KD):
                        nc.tensor.matmul(
                            ps[:], wt[:, j, ccols], hidT[:, j, cols],
                            start=(j == 0), stop=(j == KD - 1))
                    nc.vector.tensor_scalar_add(
                        dst[ct][:, cols], ps[:], bt[:, ct:ct + 1])

                def v_group(rt):
                    rows = slice(rt * P, (rt + 1) * P)
                    ps = mmp.tile([P, C], FP32, tag="mm", name="v_ps")
                    for j in range(KD):
                        nc.tensor.matmul(
                            ps[:], hidT[:, j, rows], wv[:, j, :],
                            start=(j == 0), stop=(j == KD - 1))
                    nc.vector.tensor_tensor(
                        vaug[rt][:, :, 0:HD],
                        ps.rearrange("p (h d) -> p h d", h=H), bvb[:], OP.add)
                    nc.vector.memset(vaug[rt][:, :, HD:HD + 1], 1.0)

                for ct in range(NCT):
                    for (wt, bt, dst) in ((wq, bq, qT), (wk, bk, kT)):
                        gs.append(lambda ct=ct, wt=wt, bt=bt, dst=dst:
                                  qk_group(ct, wt, bt, dst))
                for rt in range(rc * RC // P, (rc + 1) * RC // P):
                    gs.append(lambda rt=rt: v_group(rt))
                return gs

            def proj_groups(qb):
                """Output-projection groups for q block qb (list of thunks)."""
                gs = []

                def pj_group(rt, nj):
                    rows = slice(rt * P, (rt + 1) * P)
                    ncols = slice(nj * 512, (nj + 1) * 512)
                    ps = mmp.tile([P, 512], FP32, tag="mm", name="pj_ps")
                    for ct in range(NCT):
                        nc.tensor.matmul(
                            ps[:], aT[ct][:, rows], wp[:, ct, ncols],
                            start=(ct == 0), stop=(ct == NCT - 1))
                    ob = obp.tile([P, 512], FP32, tag="ob", name="ob_t")
                    nc.vector.tensor_copy(ob[:], ps[:])
                    nc.sync.dma_start(out_d[rows, ncols], ob[:])

                for rt in range(qb * QB // P, (qb + 1) * QB // P):
                    for nj in range(NPJ):
                        gs.append(lambda rt=rt, nj=nj: pj_group(rt, nj))
                return gs

            # ---- attention for one q block, with PE filler interleave ----
            def attention(qb, fillers):
                qcols = slice(qb * QB, (qb + 1) * QB)
                nkt = (qb + 1) * QB // P
                nsteps = NCT * nkt
                # spread filler groups roughly evenly over attention steps
                fill_at = {}
                for idx in range(len(fillers)):
                    pos = min(nsteps - 1, (idx * nsteps) // max(1, len(fillers)))
                    fill_at.setdefault(pos, []).append(fillers[idx])
                step = 0

                for hp in range(NCT):
                    pv2 = [pvp.tile([HD + 1, QB], FP32, tag="pv",
                                    name=f"pv_{qb}_{hp}_{i}") for i in range(2)]

                    def probs(j, qb=qb, hp=hp):
                        """scores (both heads) + one batched exp for k tile j."""
                        kcols = slice(j * P, (j + 1) * P)
                        o = (j - qb * QB // P) * P  # >=0 on diagonal tiles
                        qs = slice(qb * QB + max(o, 0), (qb + 1) * QB)
                        w = QB - max(o, 0)
                        sc2 = scp.tile([P, 2, QB], FP32, tag="sc",
                                       name=f"sc_{qb}_{hp}_{j}")
                        for i in range(2):
                            hrows = slice(i * HD, (i + 1) * HD)
                            nc.tensor.matmul(sc2[:, i, :w], kT[hp][hrows, kcols],
                                             qT[hp][hrows, qs],
                                             start=True, stop=True)
                        pt2 = ptp.tile([P, 2, QB], BF16, tag="pt", name="pt_t")
                        flat_sc = sc2.rearrange("p a b -> p (a b)")
                        flat_pt = pt2.rearrange("p a b -> p (a b)")
                        nc.scalar.activation(flat_pt[:, :QB + w], flat_sc[:, :QB + w],
                                             AF.Exp, scale=float(HD) ** -0.5)
                        if o >= 0:
                            for i in range(2):
                                nc.vector.tensor_tensor(
                                    pt2[:, i, 0:P - 1], pt2[:, i, 0:P - 1],
                                    maskb[:, 0:P - 1], OP.mult)
                        return pt2, w

                    # software pipeline: scores/exp two k tiles ahead of PV
                    PF = 2
                    pending = [probs(j) for j in range(min(PF, nkt))]
                    for j in range(nkt):
                        pt2, w = pending.pop(0)
                        if j + PF < nkt:
                            pending.append(probs(j + PF))
                        for i in range(2):
                            nc.tensor.matmul(
                                pv2[i][:, QB - w:], vaug[j][:, 2 * hp + i, :],
                                pt2[:, i, :w],
                                start=(j == 0), stop=(j == nkt - 1),
                                skip_group_check=True)
                        for f in fill_at.pop(step, ()):
                            f()
                        step += 1
                    # evacuate PV psum fast: unnormalized numerators + z row
                    for i in range(2):
                        nc.vector.tensor_copy(
                            aT[hp][i * HD:(i + 1) * HD, qcols], pv2[i][0:HD, :])
                        r = 2 * hp + i
                        nc.vector.tensor_copy(
                            zq[qb][r:r + 1, :], pv2[i][HD:HD + 1, :])
                assert not fill_at, "unplaced filler groups"

            def normalize(qb):
                qcols = slice(qb * QB, (qb + 1) * QB)
                nc.vector.reciprocal(zinv[qb][:], zq[qb][:])
                for hp in range(NCT):
                    bz2 = bzp.tile([P, QB], FP32, tag="bz", name="bz_t")
                    for i in range(2):
                        r = 2 * hp + i
                        nc.gpsimd.partition_broadcast(
                            bz2[i * HD:(i + 1) * HD, :],
                            zinv[qb][r:r + 1, :], channels=HD)
                    nc.vector.tensor_tensor(
                        aT[hp][:, qcols], aT[hp][:, qcols], bz2[:], OP.mult)

            # ---- main schedule ----
            for g in qkv_groups(0):
                g()
            for qb in range(NQB):
                fillers = []
                if qb + 1 < NQB:
                    fillers += qkv_groups(qb + 1)
                if qb > 0:
                    fillers += proj_groups(qb - 1)
                attention(qb, fillers)
                normalize(qb)
            for g in proj_groups(NQB - 1):
                g()

    nc.compile()
    return nc


def _shard_inputs(hidden_states, w_attn, b_attn, w_proj):
    bf16 = ml_dtypes.bfloat16
    in_maps = []
    for c in range(8):
        b, m = divmod(c, 2)
        sl = slice(m * C, (m + 1) * C)
        in_maps.append(dict(
            hidT=np.ascontiguousarray(hidden_states[b].T).astype(bf16),
            wq=np.ascontiguousarray(w_attn[:, sl]).astype(bf16),
            wk=np.ascontiguousarray(w_attn[:, D + m * C:D + (m + 1) * C]).astype(bf16),
            wv=np.ascontiguousarray(w_attn[:, 2 * D + m * C:2 * D + (m + 1) * C]).astype(bf16),
            bq=np.ascontiguousarray(b_attn[sl]),
            bk=np.ascontiguousarray(b_attn[D + m * C:D + (m + 1) * C]),
            bv=np.ascontiguousarray(b_attn[2 * D + m * C:2 * D + (m + 1) * C]),
            wp=np.ascontiguousarray(w_proj[sl, :]).astype(bf16),
        ))
    return in_maps


def _assemble(outs, b_proj):
    return np.stack([outs[2 * b] + outs[2 * b + 1] + b_proj[None, :]
                     for b in range(4)]).astype(np.float32)


def kernel(hidden_states, w_attn, b_attn, w_proj, b_proj):
    hidden_states = np.asarray(hidden_states, dtype=np.float32)
    w_attn = np.asarray(w_attn, dtype=np.float32)
    b_attn = np.asarray(b_attn, dtype=np.float32)
    w_proj = np.asarray(w_proj, dtype=np.float32)
    b_proj = np.asarray(b_proj, dtype=np.float32)

    if "nc" not in _NC_CACHE:
        _NC_CACHE["nc"] = _build_kernel()
    nc = _NC_CACHE["nc"]

    in_maps = _shard_inputs(hidden_states, w_attn, b_attn, w_proj)
    res = run_bass_kernel_spmd(nc, in_maps, core_ids=list(range(8)))
    outs = [r["out"] for r in res.results]
    return _assemble(outs, b_proj)


if __name__ == "__main__":
    rng = np.random.default_rng(0)
    hs = rng.standard_normal((4, S, D)).astype(np.float32)
    wa = (rng.standard_normal((D, 3 * D)) * 0.02).astype(np.float32)
    ba = np.zeros(3 * D, np.float32)
    wpj = (rng.standard_normal((D, D)) * 0.02).astype(np.float32)
    bpj = np.zeros(D, np.float32)
    out = kernel(hs, wa, ba, wpj, bpj)
    print("kernel out", out.shape, out.dtype, float(np.abs(out).max()))


# revision 13
# speedup vs baseline: 1.2358x; 1.2358x over previous
"""GPT2 causal self-attention forward on 8 Trainium2 NeuronCores (Bass/Tile).

Contract: kernel(**inputs) takes the FULL inputs of reference.setup_inputs()
  hidden_states [4, 2048, 1024] f32, w_attn [1024, 3072] f32,
  b_attn [3072] f32, w_proj [1024, 1024] f32, b_proj [1024] f32
and returns the FULL output [4, 2048, 1024] f32.

Sharding (batch x head-half, no collectives): core c = 2*b + m handles batch b
and heads m*8..m*8+7.  Each core computes q/k/v for its 8 heads (columns of
w_attn), causal attention, and a partial output projection against its 512
rows of w_proj.  The host sums the two partial projections per batch and adds
b_proj (exact, since the partials split the contraction dimension).

Per-core device pipeline (all matmuls bf16 with f32 PSUM accumulation):
  hidT [D,S] (host-pre-transposed, bf16) -> qT/kT [c,S] and V [S,c] tiles;
  scoresT[k,q] for both heads of a pair land in one 2-bank PSUM tile so a
  single scalar-engine Exp covers them; causality via column slicing on
  diagonal tiles + a triangular 128x128 band multiply; PV with a
  ones-augmented V so PSUM row 64 carries the softmax denominator.
  Normalization is deferred: unnormalized PV numerators and denominator rows
  are copied out immediately (freeing PSUM), then once per q block a single
  batched [8,512] reciprocal + gpsimd partition broadcasts + one [128,512]
  multiply per head-pair apply 1/z.  QKV / projection matmul groups are
  interleaved between attention steps so the tensor engine stays busy while
  the scalar engine streams Exp.
"""

import contextlib
import sys

for _p in ("/opt/trn_rl_repo", "/root/.axon_site/_ro/trn_rl_repo"):
    if _p not in sys.path:
        sys.path.append(_p)

import numpy as np
import ml_dtypes

import concourse.mybir as mybir
import concourse.tile as tile
from concourse import bacc
from concourse.bass_utils import run_bass_kernel_spmd

FP32 = mybir.dt.float32
BF16 = mybir.dt.bfloat16
AF = mybir.ActivationFunctionType
OP = mybir.AluOpType

S, D, H, HD = 2048, 1024, 8, 64   # per-core: sequence, model dim, heads, head dim
P = 128
C = H * HD                         # 512 channels per core
KD = D // P                        # 8 contraction tiles over D
RC, QB = 512, 512                  # row chunk / query block
NRT, NRC, NQB = S // P, S // RC, S // QB
NCT = C // P                       # 4 channel tiles (2 heads each)
NPJ = D // 512                     # 2 projection column chunks

_NC_CACHE = {}


def _build_kernel(iters=1, debug_dumps=False):
    nc = bacc.Bacc("TRN2", target_bir_lowering=False, debug=False, num_devices=8)

    hidT_d = nc.dram_tensor("hidT", [D, S], BF16, kind="ExternalInput")
    wq_d = nc.dram_tensor("wq", [D, C], BF16, kind="ExternalInput")
    wk_d = nc.dram_tensor("wk", [D, C], BF16, kind="ExternalInput")
    wv_d = nc.dram_tensor("wv", [D, C], BF16, kind="ExternalInput")
    bq_d = nc.dram_tensor("bq", [C], FP32, kind="ExternalInput")
    bk_d = nc.dram_tensor("bk", [C], FP32, kind="ExternalInput")
    bv_d = nc.dram_tensor("bv", [C], FP32, kind="ExternalInput")
    wp_d = nc.dram_tensor("wp", [C, D], BF16, kind="ExternalInput")
    out_d = nc.dram_tensor("out", [S, D], FP32, kind="ExternalOutput")

    with tile.TileContext(nc) as tc:
        with (
            tc.tile_pool(name="persist", bufs=1) as pp,
            tc.tile_pool(name="pt_pool", bufs=6) as ptp,
            tc.tile_pool(name="work", bufs=2) as wkp,
            tc.tile_pool(name="bzp", bufs=2) as bzp,
            tc.tile_pool(name="un", bufs=10) as unp,
            tc.tile_pool(name="outp", bufs=3) as obp,
            tc.tile_pool(name="mm", bufs=2, space="PSUM") as mmp,
            tc.tile_pool(name="sc", bufs=2, space="PSUM") as scp,
            tc.tile_pool(name="pv", bufs=2, space="PSUM") as pvp,
        ):
          with (tc.For_i(0, iters) if iters > 1 else contextlib.nullcontext()):
            # ---- persistent tiles ----
            hidT = pp.tile([P, KD, S], BF16, tag="hidT", name="hidT_sb")
            wq = pp.tile([P, KD, C], BF16, tag="wq", name="wq_sb")
            wk = pp.tile([P, KD, C], BF16, tag="wk", name="wk_sb")
            wv = pp.tile([P, KD, C], BF16, tag="wv", name="wv_sb")
            wp = pp.tile([P, NCT, D], BF16, tag="wp", name="wp_sb")
            bq = pp.tile([P, NCT], FP32, tag="bq", name="bq_sb")
            bk = pp.tile([P, NCT], FP32, tag="bk", name="bk_sb")
            bv1 = pp.tile([1, C], FP32, tag="bv1", name="bv1_sb")
            bvb = pp.tile([P, H, HD], FP32, tag="bvb", name="bvb_sb")
            maskf = pp.tile([P, P], FP32, tag="maskf", name="maskf_sb")
            maskb = pp.tile([P, P], BF16, tag="maskb", name="maskb_sb")
            qT = [pp.tile([P, S], BF16, tag=f"qT{ct}", name=f"qT{ct}_sb")
                  for ct in range(NCT)]
            kT = [pp.tile([P, S], BF16, tag=f"kT{ct}", name=f"kT{ct}_sb")
                  for ct in range(NCT)]
            vaug = [pp.tile([P, H, HD + 1], BF16, tag=f"va{rt}", name=f"va{rt}_sb")
                    for rt in range(NRT)]
            aT = [pp.tile([P, S], BF16, tag=f"aT{ct}", name=f"aT{ct}_sb")
                  for ct in range(NCT)]
            # z rows live at quadrant-aligned partitions {0,32,64,96} (the only
            # legal single-partition access bases): tile [2][i] holds head-pair
            # hp's denominator for head i at partition 32*hp.
            zq = [[pp.tile([P, QB], FP32, tag=f"zq{qb}_{i}", name=f"zq{qb}{i}_sb")
                   for i in range(2)] for qb in range(NQB)]
            zinv = [[pp.tile([P, QB], FP32, tag=f"zi{qb}_{i}", name=f"zi{qb}{i}_sb")
                     for i in range(2)] for qb in range(NQB)]

            # ---- input DMA ----
            for j in range(KD):
                nc.sync.dma_start(hidT[:, j, :], hidT_d[j * P:(j + 1) * P, :])
                nc.sync.dma_start(wq[:, j, :], wq_d[j * P:(j + 1) * P, :])
                nc.sync.dma_start(wk[:, j, :], wk_d[j * P:(j + 1) * P, :])
                nc.sync.dma_start(wv[:, j, :], wv_d[j * P:(j + 1) * P, :])
            for ct in range(NCT):
                nc.sync.dma_start(wp[:, ct, :], wp_d[ct * P:(ct + 1) * P, :])
            nc.sync.dma_start(bq[:], bq_d.rearrange("(ct p) -> p ct", p=P))
            nc.sync.dma_start(bk[:], bk_d.rearrange("(ct p) -> p ct", p=P))
            nc.sync.dma_start(bv1[:], bv_d[None, :])

            # broadcast bv across partitions: bvb[p, h, dd] = bv[64h+dd]
            nc.gpsimd.partition_broadcast(
                bvb.rearrange("p h d -> p (h d)"), bv1[:], channels=P)

            # triangular mask band: maskb[kk, u] = 1 if u >= kk else 0
            nc.vector.memset(maskf[:], 1.0)
            nc.gpsimd.affine_select(
                out=maskf[:], in_=maskf[:], compare_op=OP.is_ge, fill=0.0,
                base=0, channel_multiplier=-1, pattern=[[1, P]])
            nc.vector.tensor_copy(maskb[:], maskf[:])

            # ---- PE work generators ----
            def qkv_groups(rc):
                """QKV matmul groups for sequence chunk rc (list of thunks)."""
                gs = []
                cols = slice(rc * RC, (rc + 1) * RC)

                def qk_group(ct, wt, bt, dst):
                    ccols = slice(ct * P, (ct + 1) * P)
                    ps = mmp.tile([P, RC], FP32, tag="mm", name="qk_ps")
                    for j in range(KD):
                        nc.tensor.matmul(
                            ps[:], wt[:, j, ccols], hidT[:, j, cols],
                            start=(j == 0), stop=(j == KD - 1))
                    nc.vector.tensor_scalar_add(
                        dst[ct][:, cols], ps[:], bt[:, ct:ct + 1])

                def v_group(rt):
                    rows = slice(rt * P, (rt + 1) * P)
                    ps = mmp.tile([P, C], FP32, tag="mm", name="v_ps")
                    for j in range(KD):
                        nc.tensor.matmul(
                            ps[:], hidT[:, j, rows], wv[:, j, :],
                            start=(j == 0), stop=(j == KD - 1))
                    nc.vector.tensor_tensor(
                        vaug[rt][:, :, 0:HD],
                        ps.rearrange("p (h d) -> p h d", h=H), bvb[:], OP.add)
                    nc.vector.memset(vaug[rt][:, :, HD:HD + 1], 1.0)

                for ct in range(NCT):
                    for (wt, bt, dst) in ((wq, bq, qT), (wk, bk, kT)):
                        gs.append(lambda ct=ct, wt=wt, bt=bt, dst=dst:
                                  qk_group(ct, wt, bt, dst))
                for rt in range(rc * RC // P, (rc + 1) * RC // P):
                    gs.append(lambda rt=rt: v_group(rt))
                return gs

            def proj_groups(qb):
                """Output-projection groups for q block qb (list of thunks)."""
                gs = []

                def pj_group(rt, nj):
                    rows = slice(rt * P, (rt + 1) * P)
                    ncols = slice(nj * 512, (nj + 1) * 512)
                    ps = mmp.tile([P, 512], FP32, tag="mm", name="pj_ps")
                    for ct in range(NCT):
                        nc.tensor.matmul(
                            ps[:], aT[ct][:, rows], wp[:, ct, ncols],
                            start=(ct == 0), stop=(ct == NCT - 1))
                    ob = obp.tile([P, 512], FP32, tag="ob", name="ob_t")
                    nc.vector.tensor_copy(ob[:], ps[:])
                    nc.sync.dma_start(out_d[rows, ncols], ob[:])

                for rt in range(qb * QB // P, (qb + 1) * QB // P):
                    for nj in range(NPJ):
                        gs.append(lambda rt=rt, nj=nj: pj_group(rt, nj))
                return gs

            # ---- attention for one q block, with PE filler interleave ----
            def attention(qb, fillers, uns):
                qcols = slice(qb * QB, (qb + 1) * QB)
                nkt = (qb + 1) * QB // P
                nsteps = NCT * nkt
                # spread filler groups roughly evenly over attention steps
                fill_at = {}
                for idx in range(len(fillers)):
                    pos = min(nsteps - 1, (idx * nsteps) // max(1, len(fillers)))
                    fill_at.setdefault(pos, []).append(fillers[idx])
                step = 0

                for hp in range(NCT):
                    pv2 = [pvp.tile([HD + 1, QB], FP32, tag="pv",
                                    name=f"pv_{qb}_{hp}_{i}") for i in range(2)]

                    def probs(j, qb=qb, hp=hp):
                        """scores (both heads) + one batched exp for k tile j."""
                        kcols = slice(j * P, (j + 1) * P)
                        o = (j - qb * QB // P) * P  # >=0 on diagonal tiles
                        qs = slice(qb * QB + max(o, 0), (qb + 1) * QB)
                        w = QB - max(o, 0)
                        sc2 = scp.tile([P, 2, QB], FP32, tag="sc",
                                       name=f"sc_{qb}_{hp}_{j}")
                        for i in range(2):
                            hrows = slice(i * HD, (i + 1) * HD)
                            nc.tensor.matmul(sc2[:, i, :w], kT[hp][hrows, kcols],
                                             qT[hp][hrows, qs],
                                             start=True, stop=True)
                        pt2 = ptp.tile([P, 2, QB], BF16, tag="pt", name="pt_t")
                        flat_sc = sc2.rearrange("p a b -> p (a b)")
                        flat_pt = pt2.rearrange("p a b -> p (a b)")
                        nc.scalar.activation(flat_pt[:, :QB + w], flat_sc[:, :QB + w],
                                             AF.Exp, scale=float(HD) ** -0.5)
                        if o >= 0:
                            for i in range(2):
                                nc.vector.tensor_tensor(
                                    pt2[:, i, 0:P - 1], pt2[:, i, 0:P - 1],
                                    maskb[:, 0:P - 1], OP.mult)
                        return pt2, w

                    # software pipeline: scores/exp two k tiles ahead of PV
                    PF = 2
                    pending = [probs(j) for j in range(min(PF, nkt))]
                    for j in range(nkt):
                        pt2, w = pending.pop(0)
                        if j + PF < nkt:
                            pending.append(probs(j + PF))
                        for i in range(2):
                            nc.tensor.matmul(
                                pv2[i][:, QB - w:], vaug[j][:, 2 * hp + i, :],
                                pt2[:, i, :w],
                                start=(j == 0), stop=(j == nkt - 1),
                                skip_group_check=True)
                        for f in fill_at.pop(step, ()):
                            f()
                        step += 1
                    # evacuate PV psum fast: unnormalized numerators into
                    # base-0 staging (TT inputs must share a base partition,
                    # so the normalize multiply needs them at partition 0),
                    # plus the denominator row into its quadrant slot.
                    for i in range(2):
                        un = unp.tile([HD, QB], BF16, tag="un",
                                      name=f"un_{qb}_{hp}_{i}")
                        nc.vector.tensor_copy(un[:], pv2[i][0:HD, :])
                        uns[2 * hp + i] = un
                        nc.vector.tensor_copy(
                            zq[qb][i][32 * hp:32 * hp + 1, :],
                            pv2[i][HD:HD + 1, :])
                assert not fill_at, "unplaced filler groups"

            def normalize(qb, uns):
                # partition_broadcast only works from a base-0 source (HW
                # ucode limitation, verified) -> stage each 1/z row at
                # partition 0 first.
                qcols = slice(qb * QB, (qb + 1) * QB)
                for i in range(2):
                    nc.vector.reciprocal(zinv[qb][i][:], zq[qb][i][:])
                for hp in range(NCT):
                    for i in range(2):
                        zrow = wkp.tile([1, QB], FP32, tag="zrow", name="zrow_t")
                        nc.vector.tensor_copy(
                            zrow[:], zinv[qb][i][32 * hp:32 * hp + 1, :])
                        bz = bzp.tile([HD, QB], FP32, tag="bz", name="bz_t")
                        nc.gpsimd.partition_broadcast(bz[:], zrow[:], channels=HD)
                        nc.vector.tensor_tensor(
                            aT[hp][i * HD:(i + 1) * HD, qcols],
                            uns[2 * hp + i][:], bz[:], OP.mult)

            # ---- main schedule ----
            for g in qkv_groups(0):
                g()
            for qb in range(NQB):
                fillers = []
                if qb + 1 < NQB:
                    fillers += qkv_groups(qb + 1)
                if qb > 0:
                    fillers += proj_groups(qb - 1)
                uns = [None] * H
                attention(qb, fillers, uns)
                normalize(qb, uns)
            for g in proj_groups(NQB - 1):
                g()

    nc.compile()
    return nc


def _shard_inputs(hidden_states, w_attn, b_attn, w_proj):
    bf16 = ml_dtypes.bfloat16
    in_maps = []
    for c in range(8):
        b, m = divmod(c, 2)
        sl = slice(m * C, (m + 1) * C)
        in_maps.append(dict(
            hidT=np.ascontiguousarray(hidden_states[b].T).astype(bf16),
            wq=np.ascontiguousarray(w_attn[:, sl]).astype(bf16),
            wk=np.ascontiguousarray(w_attn[:, D + m * C:D + (m + 1) * C]).astype(bf16),
            wv=np.ascontiguousarray(w_attn[:, 2 * D + m * C:2 * D + (m + 1) * C]).astype(bf16),
            bq=np.ascontiguousarray(b_attn[sl]),
            bk=np.ascontiguousarray(b_attn[D + m * C:D + (m + 1) * C]),
            bv=np.ascontiguousarray(b_attn[2 * D + m * C:2 * D + (m + 1) * C]),
            wp=np.ascontiguousarray(w_proj[sl, :]).astype(bf16),
        ))
    return in_maps


def _assemble(outs, b_proj):
    return np.stack([outs[2 * b] + outs[2 * b + 1] + b_proj[None, :]
                     for b in range(4)]).astype(np.float32)


def kernel(hidden_states, w_attn, b_attn, w_proj, b_proj):
    hidden_states = np.asarray(hidden_states, dtype=np.float32)
    w_attn = np.asarray(w_attn, dtype=np.float32)
    b_attn = np.asarray(b_attn, dtype=np.float32)
    w_proj = np.asarray(w_proj, dtype=np.float32)
    b_proj = np.asarray(b_proj, dtype=np.float32)

    if "nc" not in _NC_CACHE:
        _NC_CACHE["nc"] = _build_kernel()
    nc = _NC_CACHE["nc"]

    in_maps = _shard_inputs(hidden_states, w_attn, b_attn, w_proj)
    res = run_bass_kernel_spmd(nc, in_maps, core_ids=list(range(8)))
    outs = [r["out"] for r in res.results]
    return _assemble(outs, b_proj)


if __name__ == "__main__":
    rng = np.random.default_rng(0)
    hs = rng.standard_normal((4, S, D)).astype(np.float32)
    wa = (rng.standard_normal((D, 3 * D)) * 0.02).astype(np.float32)
    ba = np.zeros(3 * D, np.float32)
    wpj = (rng.standard_normal((D, D)) * 0.02).astype(np.float32)
    bpj = np.zeros(D, np.float32)
    out = kernel(hs, wa, ba, wpj, bpj)
    print("kernel out", out.shape, out.dtype, float(np.abs(out).max()))


# revision 15
# speedup vs baseline: 1.4419x; 1.1668x over previous
"""GPT2 causal self-attention forward on 8 Trainium2 NeuronCores (Bass/Tile).

Contract: kernel(**inputs) takes the FULL inputs of reference.setup_inputs()
  hidden_states [4, 2048, 1024] f32, w_attn [1024, 3072] f32,
  b_attn [3072] f32, w_proj [1024, 1024] f32, b_proj [1024] f32
and returns the FULL output [4, 2048, 1024] f32.

Sharding (batch x head-half, no collectives): core c = 2*b + m handles batch b
and heads m*8..m*8+7.  Each core computes q/k/v for its 8 heads (columns of
w_attn), causal attention, and a partial output projection against its 512
rows of w_proj.  The host sums the two partial projections per batch and adds
b_proj (exact, since the partials split the contraction dimension).

Per-core device pipeline (all matmuls bf16 with f32 PSUM accumulation):
  hidT [D,S] (host-pre-transposed, bf16) -> qT/kT [c,S] and V [S,c] tiles;
  scoresT[k,q] for both heads of a pair land in one 2-bank PSUM tile so a
  single scalar-engine Exp covers them; causality via column slicing on
  diagonal tiles + a triangular 128x128 band multiply; PV with a
  ones-augmented V so PSUM row 64 carries the softmax denominator.
  Normalization is deferred: unnormalized PV numerators and denominator rows
  are copied out immediately (freeing PSUM), then once per q block a single
  batched [8,512] reciprocal + gpsimd partition broadcasts + one [128,512]
  multiply per head-pair apply 1/z.  QKV / projection matmul groups are
  interleaved between attention steps so the tensor engine stays busy while
  the scalar engine streams Exp.
"""

import contextlib
import sys

for _p in ("/opt/trn_rl_repo", "/root/.axon_site/_ro/trn_rl_repo"):
    if _p not in sys.path:
        sys.path.append(_p)

import numpy as np
import ml_dtypes

import concourse.mybir as mybir
import concourse.tile as tile
from concourse import bacc
from concourse.bass_utils import run_bass_kernel_spmd

FP32 = mybir.dt.float32
BF16 = mybir.dt.bfloat16
AF = mybir.ActivationFunctionType
OP = mybir.AluOpType

S, D, H, HD = 2048, 1024, 8, 64   # per-core: sequence, model dim, heads, head dim
P = 128
C = H * HD                         # 512 channels per core
KD = D // P                        # 8 contraction tiles over D
RC, QB = 512, 512                  # row chunk / query block
NRT, NRC, NQB = S // P, S // RC, S // QB
NCT = C // P                       # 4 channel tiles (2 heads each)
NPJ = D // 512                     # 2 projection column chunks

_NC_CACHE = {}


def _build_kernel(iters=1, debug_dumps=False):
    nc = bacc.Bacc("TRN2", target_bir_lowering=False, debug=False, num_devices=8)

    hidT_d = nc.dram_tensor("hidT", [D, S], BF16, kind="ExternalInput")
    wq_d = nc.dram_tensor("wq", [D, C], BF16, kind="ExternalInput")
    wk_d = nc.dram_tensor("wk", [D, C], BF16, kind="ExternalInput")
    wv_d = nc.dram_tensor("wv", [D, C], BF16, kind="ExternalInput")
    bq_d = nc.dram_tensor("bq", [C], FP32, kind="ExternalInput")
    bk_d = nc.dram_tensor("bk", [C], FP32, kind="ExternalInput")
    bv_d = nc.dram_tensor("bv", [C], FP32, kind="ExternalInput")
    wp_d = nc.dram_tensor("wp", [C, D], BF16, kind="ExternalInput")
    out_d = nc.dram_tensor("out", [S, D], FP32, kind="ExternalOutput")

    with tile.TileContext(nc) as tc:
        with (
            tc.tile_pool(name="persist", bufs=1) as pp,
            tc.tile_pool(name="pt_pool", bufs=6) as ptp,
            tc.tile_pool(name="work", bufs=2) as wkp,
            tc.tile_pool(name="bzp", bufs=2) as bzp,
            tc.tile_pool(name="un", bufs=10) as unp,
            tc.tile_pool(name="outp", bufs=3) as obp,
            tc.tile_pool(name="mm", bufs=2, space="PSUM") as mmp,
            tc.tile_pool(name="sc", bufs=2, space="PSUM") as scp,
            tc.tile_pool(name="pv", bufs=2, space="PSUM") as pvp,
        ):
          with (tc.For_i(0, iters) if iters > 1 else contextlib.nullcontext()):
            # ---- persistent tiles ----
            hidT = pp.tile([P, KD, S], BF16, tag="hidT", name="hidT_sb")
            wq = pp.tile([P, KD, C], BF16, tag="wq", name="wq_sb")
            wk = pp.tile([P, KD, C], BF16, tag="wk", name="wk_sb")
            wv = pp.tile([P, KD, C], BF16, tag="wv", name="wv_sb")
            wp = pp.tile([P, NCT, D], BF16, tag="wp", name="wp_sb")
            bq = pp.tile([P, NCT], FP32, tag="bq", name="bq_sb")
            bk = pp.tile([P, NCT], FP32, tag="bk", name="bk_sb")
            bv1 = pp.tile([1, C], FP32, tag="bv1", name="bv1_sb")
            bvb = pp.tile([P, H, HD], FP32, tag="bvb", name="bvb_sb")
            maskf = pp.tile([P, P], FP32, tag="maskf", name="maskf_sb")
            maskb = pp.tile([P, P], BF16, tag="maskb", name="maskb_sb")
            qT = [pp.tile([P, S], BF16, tag=f"qT{ct}", name=f"qT{ct}_sb")
                  for ct in range(NCT)]
            kT = [pp.tile([P, S], BF16, tag=f"kT{ct}", name=f"kT{ct}_sb")
                  for ct in range(NCT)]
            vaug = [pp.tile([P, H, HD + 1], BF16, tag=f"va{rt}", name=f"va{rt}_sb")
                    for rt in range(NRT)]
            aT = [pp.tile([P, S], BF16, tag=f"aT{ct}", name=f"aT{ct}_sb")
                  for ct in range(NCT)]
            # z rows live at quadrant-aligned partitions {0,32,64,96} (the only
            # legal single-partition access bases): tile [2][i] holds head-pair
            # hp's denominator for head i at partition 32*hp.
            zq = [[pp.tile([P, QB], FP32, tag=f"zq{qb}_{i}", name=f"zq{qb}{i}_sb")
                   for i in range(2)] for qb in range(NQB)]
            zinv = [[pp.tile([P, QB], FP32, tag=f"zi{qb}_{i}", name=f"zi{qb}{i}_sb")
                     for i in range(2)] for qb in range(NQB)]

            # ---- input DMA (few big transfers: per-DMA overhead dominates) ----
            HK = KD // 2
            nc.sync.dma_start(
                hidT[:, 0:HK, :],
                hidT_d[0:HK * P, :].rearrange("(j p) s -> p j s", p=P))
            nc.sync.dma_start(
                hidT[:, HK:KD, :],
                hidT_d[HK * P:KD * P, :].rearrange("(j p) s -> p j s", p=P))
            nc.sync.dma_start(wq[:], wq_d.rearrange("(j p) c -> p j c", p=P))
            nc.sync.dma_start(wk[:], wk_d.rearrange("(j p) c -> p j c", p=P))
            nc.sync.dma_start(wv[:], wv_d.rearrange("(j p) c -> p j c", p=P))
            nc.sync.dma_start(wp[:], wp_d.rearrange("(ct p) d -> p ct d", p=P))
            nc.sync.dma_start(bq[:], bq_d.rearrange("(ct p) -> p ct", p=P))
            nc.sync.dma_start(bk[:], bk_d.rearrange("(ct p) -> p ct", p=P))
            nc.sync.dma_start(bv1[:], bv_d[None, :])

            # broadcast bv across partitions: bvb[p, h, dd] = bv[64h+dd]
            nc.gpsimd.partition_broadcast(
                bvb.rearrange("p h d -> p (h d)"), bv1[:], channels=P)

            # triangular mask band: maskb[kk, u] = 1 if u >= kk else 0
            nc.vector.memset(maskf[:], 1.0)
            nc.gpsimd.affine_select(
                out=maskf[:], in_=maskf[:], compare_op=OP.is_ge, fill=0.0,
                base=0, channel_multiplier=-1, pattern=[[1, P]])
            nc.vector.tensor_copy(maskb[:], maskf[:])

            # ---- PE work generators ----
            def qkv_groups(rc):
                """QKV matmul groups for sequence chunk rc (list of thunks)."""
                gs = []
                cols = slice(rc * RC, (rc + 1) * RC)

                def qk_group(ct, wt, bt, dst):
                    ccols = slice(ct * P, (ct + 1) * P)
                    ps = mmp.tile([P, RC], FP32, tag="mm", name="qk_ps")
                    for j in range(KD):
                        nc.tensor.matmul(
                            ps[:], wt[:, j, ccols], hidT[:, j, cols],
                            start=(j == 0), stop=(j == KD - 1))
                    nc.vector.tensor_scalar_add(
                        dst[ct][:, cols], ps[:], bt[:, ct:ct + 1])

                def v_group(rt):
                    rows = slice(rt * P, (rt + 1) * P)
                    ps = mmp.tile([P, C], FP32, tag="mm", name="v_ps")
                    for j in range(KD):
                        nc.tensor.matmul(
                            ps[:], hidT[:, j, rows], wv[:, j, :],
                            start=(j == 0), stop=(j == KD - 1))
                    nc.vector.tensor_tensor(
                        vaug[rt][:, :, 0:HD],
                        ps.rearrange("p (h d) -> p h d", h=H), bvb[:], OP.add)
                    nc.vector.memset(vaug[rt][:, :, HD:HD + 1], 1.0)

                for ct in range(NCT):
                    for (wt, bt, dst) in ((wq, bq, qT), (wk, bk, kT)):
                        gs.append(lambda ct=ct, wt=wt, bt=bt, dst=dst:
                                  qk_group(ct, wt, bt, dst))
                for rt in range(rc * RC // P, (rc + 1) * RC // P):
                    gs.append(lambda rt=rt: v_group(rt))
                return gs

            def proj_groups(qb):
                """Output-projection groups for q block qb (list of thunks)."""
                gs = []

                def pj_group(rt):
                    rows = slice(rt * P, (rt + 1) * P)
                    ob = obp.tile([P, NPJ, 512], FP32, tag="ob", name="ob_t")
                    for nj in range(NPJ):
                        ncols = slice(nj * 512, (nj + 1) * 512)
                        ps = mmp.tile([P, 512], FP32, tag="mm", name="pj_ps")
                        for ct in range(NCT):
                            nc.tensor.matmul(
                                ps[:], aT[ct][:, rows], wp[:, ct, ncols],
                                start=(ct == 0), stop=(ct == NCT - 1))
                        nc.vector.tensor_copy(ob[:, nj, :], ps[:])
                    nc.sync.dma_start(out_d[rows, :],
                                      ob.rearrange("p a b -> p (a b)"))

                for rt in range(qb * QB // P, (qb + 1) * QB // P):
                    gs.append(lambda rt=rt: pj_group(rt))
                return gs

            # ---- attention for one q block, with PE filler interleave ----
            def attention(qb, fillers, uns):
                qcols = slice(qb * QB, (qb + 1) * QB)
                nkt = (qb + 1) * QB // P
                nsteps = NCT * nkt
                # spread filler groups roughly evenly over attention steps
                fill_at = {}
                for idx in range(len(fillers)):
                    pos = min(nsteps - 1, (idx * nsteps) // max(1, len(fillers)))
                    fill_at.setdefault(pos, []).append(fillers[idx])
                step = 0

                for hp in range(NCT):
                    pv2 = [pvp.tile([HD + 1, QB], FP32, tag="pv",
                                    name=f"pv_{qb}_{hp}_{i}") for i in range(2)]

                    def probs(j, qb=qb, hp=hp):
                        """scores (both heads) + one batched exp for k tile j."""
                        kcols = slice(j * P, (j + 1) * P)
                        o = (j - qb * QB // P) * P  # >=0 on diagonal tiles
                        qs = slice(qb * QB + max(o, 0), (qb + 1) * QB)
                        w = QB - max(o, 0)
                        sc2 = scp.tile([P, 2, QB], FP32, tag="sc",
                                       name=f"sc_{qb}_{hp}_{j}")
                        for i in range(2):
                            hrows = slice(i * HD, (i + 1) * HD)
                            nc.tensor.matmul(sc2[:, i, :w], kT[hp][hrows, kcols],
                                             qT[hp][hrows, qs],
                                             start=True, stop=True)
                        pt2 = ptp.tile([P, 2, QB], BF16, tag="pt", name="pt_t")
                        flat_sc = sc2.rearrange("p a b -> p (a b)")
                        flat_pt = pt2.rearrange("p a b -> p (a b)")
                        nc.scalar.activation(flat_pt[:, :QB + w], flat_sc[:, :QB + w],
                                             AF.Exp, scale=float(HD) ** -0.5)
                        if o >= 0:
                            for i in range(2):
                                nc.vector.tensor_tensor(
                                    pt2[:, i, 0:P - 1], pt2[:, i, 0:P - 1],
                                    maskb[:, 0:P - 1], OP.mult)
                        return pt2, w

                    # software pipeline: scores/exp two k tiles ahead of PV
                    PF = 2
                    pending = [probs(j) for j in range(min(PF, nkt))]
                    for j in range(nkt):
                        pt2, w = pending.pop(0)
                        if j + PF < nkt:
                            pending.append(probs(j + PF))
                        for i in range(2):
                            nc.tensor.matmul(
                                pv2[i][:, QB - w:], vaug[j][:, 2 * hp + i, :],
                                pt2[:, i, :w],
                                start=(j == 0), stop=(j == nkt - 1),
                                skip_group_check=True)
                        for f in fill_at.pop(step, ()):
                            f()
                        step += 1
                    # evacuate PV psum fast: unnormalized numerators into
                    # base-0 staging (TT inputs must share a base partition,
                    # so the normalize multiply needs them at partition 0),
                    # plus the denominator row into its quadrant slot.
                    for i in range(2):
                        un = unp.tile([HD, QB], BF16, tag="un",
                                      name=f"un_{qb}_{hp}_{i}")
                        nc.vector.tensor_copy(un[:], pv2[i][0:HD, :])
                        uns[2 * hp + i] = un
                        nc.vector.tensor_copy(
                            zq[qb][i][32 * hp:32 * hp + 1, :],
                            pv2[i][HD:HD + 1, :])
                assert not fill_at, "unplaced filler groups"

            def normalize(qb, uns):
                # partition_broadcast only works from a base-0 source (HW
                # ucode limitation, verified) -> stage each 1/z row at
                # partition 0 first.
                qcols = slice(qb * QB, (qb + 1) * QB)
                for i in range(2):
                    nc.vector.reciprocal(zinv[qb][i][:], zq[qb][i][:])
                for hp in range(NCT):
                    for i in range(2):
                        zrow = wkp.tile([1, QB], FP32, tag="zrow", name="zrow_t")
                        nc.vector.tensor_copy(
                            zrow[:], zinv[qb][i][32 * hp:32 * hp + 1, :])
                        bz = bzp.tile([HD, QB], FP32, tag="bz", name="bz_t")
                        nc.gpsimd.partition_broadcast(bz[:], zrow[:], channels=HD)
                        nc.vector.tensor_tensor(
                            aT[hp][i * HD:(i + 1) * HD, qcols],
                            uns[2 * hp + i][:], bz[:], OP.mult)

            # ---- main schedule ----
            for g in qkv_groups(0):
                g()
            for qb in range(NQB):
                fillers = []
                if qb + 1 < NQB:
                    fillers += qkv_groups(qb + 1)
                if qb > 0:
                    fillers += proj_groups(qb - 1)
                uns = [None] * H
                attention(qb, fillers, uns)
                normalize(qb, uns)
            for g in proj_groups(NQB - 1):
                g()

    nc.compile()
    return nc


def _shard_inputs(hidden_states, w_attn, b_attn, w_proj):
    bf16 = ml_dtypes.bfloat16
    in_maps = []
    for c in range(8):
        b, m = divmod(c, 2)
        sl = slice(m * C, (m + 1) * C)
        in_maps.append(dict(
            hidT=np.ascontiguousarray(hidden_states[b].T).astype(bf16),
            wq=np.ascontiguousarray(w_attn[:, sl]).astype(bf16),
            wk=np.ascontiguousarray(w_attn[:, D + m * C:D + (m + 1) * C]).astype(bf16),
            wv=np.ascontiguousarray(w_attn[:, 2 * D + m * C:2 * D + (m + 1) * C]).astype(bf16),
            bq=np.ascontiguousarray(b_attn[sl]),
            bk=np.ascontiguousarray(b_attn[D + m * C:D + (m + 1) * C]),
            bv=np.ascontiguousarray(b_attn[2 * D + m * C:2 * D + (m + 1) * C]),
            wp=np.ascontiguousarray(w_proj[sl, :]).astype(bf16),
        ))
    return in_maps


def _assemble(outs, b_proj):
    return np.stack([outs[2 * b] + outs[2 * b + 1] + b_proj[None, :]
                     for b in range(4)]).astype(np.float32)


def kernel(hidden_states, w_attn, b_attn, w_proj, b_proj):
    hidden_states = np.asarray(hidden_states, dtype=np.float32)
    w_attn = np.asarray(w_attn, dtype=np.float32)
    b_attn = np.asarray(b_attn, dtype=np.float32)
    w_proj = np.asarray(w_proj, dtype=np.float32)
    b_proj = np.asarray(b_proj, dtype=np.float32)

    if "nc" not in _NC_CACHE:
        _NC_CACHE["nc"] = _build_kernel()
    nc = _NC_CACHE["nc"]

    in_maps = _shard_inputs(hidden_states, w_attn, b_attn, w_proj)
    res = run_bass_kernel_spmd(nc, in_maps, core_ids=list(range(8)))
    outs = [r["out"] for r in res.results]
    return _assemble(outs, b_proj)


if __name__ == "__main__":
    rng = np.random.default_rng(0)
    hs = rng.standard_normal((4, S, D)).astype(np.float32)
    wa = (rng.standard_normal((D, 3 * D)) * 0.02).astype(np.float32)
    ba = np.zeros(3 * D, np.float32)
    wpj = (rng.standard_normal((D, D)) * 0.02).astype(np.float32)
    bpj = np.zeros(D, np.float32)
    out = kernel(hs, wa, ba, wpj, bpj)
    print("kernel out", out.shape, out.dtype, float(np.abs(out).max()))


# revision 18
# speedup vs baseline: 1.6416x; 1.1385x over previous
"""GPT2 causal self-attention forward on 8 Trainium2 NeuronCores (Bass/Tile).

Contract: kernel(**inputs) takes the FULL inputs of reference.setup_inputs()
  hidden_states [4, 2048, 1024] f32, w_attn [1024, 3072] f32,
  b_attn [3072] f32, w_proj [1024, 1024] f32, b_proj [1024] f32
and returns the FULL output [4, 2048, 1024] f32.

Sharding (batch x head-half, no collectives): core c = 2*b + m handles batch b
and heads m*8..m*8+7.  Each core computes q/k/v for its 8 heads (columns of
w_attn), causal attention, and a partial output projection against its 512
rows of w_proj.  The host sums the two partial projections per batch and adds
b_proj (exact, since the partials split the contraction dimension).

Per-core device pipeline (all matmuls bf16 with f32 PSUM accumulation):
  hidT [D,S] (host-pre-transposed, bf16) -> qT/kT [c,S] and V [S,c] tiles;
  scoresT[k,q] for both heads of a pair land in one 2-bank PSUM tile so a
  single scalar-engine Exp covers them; causality via column slicing on
  diagonal tiles + a triangular 128x128 band multiply; PV with a
  ones-augmented V so PSUM row 64 carries the softmax denominator.
  Normalization is deferred: unnormalized PV numerators and denominator rows
  are copied out immediately (freeing PSUM), then once per q block a single
  batched [8,512] reciprocal + gpsimd partition broadcasts + one [128,512]
  multiply per head-pair apply 1/z.  QKV / projection matmul groups are
  interleaved between attention steps so the tensor engine stays busy while
  the scalar engine streams Exp.
"""

import contextlib
import sys

for _p in ("/opt/trn_rl_repo", "/root/.axon_site/_ro/trn_rl_repo"):
    if _p not in sys.path:
        sys.path.append(_p)

import numpy as np
import ml_dtypes

import concourse.mybir as mybir
import concourse.tile as tile
from concourse import bacc
from concourse.bass_utils import run_bass_kernel_spmd

FP32 = mybir.dt.float32
BF16 = mybir.dt.bfloat16
AF = mybir.ActivationFunctionType
OP = mybir.AluOpType

S, D, H, HD = 2048, 1024, 8, 64   # per-core: sequence, model dim, heads, head dim
P = 128
C = H * HD                         # 512 channels per core
KD = D // P                        # 8 contraction tiles over D
RC, QB = 512, 512                  # row chunk / query block
NRT, NRC, NQB = S // P, S // RC, S // QB
NCT = C // P                       # 4 channel tiles (2 heads each)
NPJ = D // 512                     # 2 projection column chunks

_NC_CACHE = {}


def _build_kernel(iters=1, debug_dumps=False):
    nc = bacc.Bacc("TRN2", target_bir_lowering=False, debug=False, num_devices=8)

    hidT_d = nc.dram_tensor("hidT", [D, S], BF16, kind="ExternalInput")
    wq_d = nc.dram_tensor("wq", [D, C], BF16, kind="ExternalInput")
    wk_d = nc.dram_tensor("wk", [D, C], BF16, kind="ExternalInput")
    wv_d = nc.dram_tensor("wv", [D, C], BF16, kind="ExternalInput")
    bq_d = nc.dram_tensor("bq", [C], FP32, kind="ExternalInput")
    bk_d = nc.dram_tensor("bk", [C], FP32, kind="ExternalInput")
    bv_d = nc.dram_tensor("bv", [C], FP32, kind="ExternalInput")
    wp_d = nc.dram_tensor("wp", [C, D], BF16, kind="ExternalInput")
    out_d = nc.dram_tensor("out", [S, D], FP32, kind="ExternalOutput")

    with tile.TileContext(nc) as tc:
        with (
            tc.tile_pool(name="persist", bufs=1) as pp,
            tc.tile_pool(name="pt_pool", bufs=6) as ptp,
            tc.tile_pool(name="work", bufs=2) as wkp,
            tc.tile_pool(name="bzp", bufs=2) as bzp,
            tc.tile_pool(name="un", bufs=10) as unp,
            tc.tile_pool(name="outp", bufs=3) as obp,
            tc.tile_pool(name="mm", bufs=2, space="PSUM") as mmp,
            tc.tile_pool(name="sc", bufs=2, space="PSUM") as scp,
            tc.tile_pool(name="pv", bufs=2, space="PSUM") as pvp,
        ):
          with (tc.For_i(0, iters) if iters > 1 else contextlib.nullcontext()):
            # ---- persistent tiles ----
            hidT = pp.tile([P, KD, S], BF16, tag="hidT", name="hidT_sb")
            wq = pp.tile([P, KD, C], BF16, tag="wq", name="wq_sb")
            wk = pp.tile([P, KD, C], BF16, tag="wk", name="wk_sb")
            wv = pp.tile([P, KD, C], BF16, tag="wv", name="wv_sb")
            wp = pp.tile([P, NCT, D], BF16, tag="wp", name="wp_sb")
            bq = pp.tile([P, NCT], FP32, tag="bq", name="bq_sb")
            bk = pp.tile([P, NCT], FP32, tag="bk", name="bk_sb")
            bv1 = pp.tile([1, C], FP32, tag="bv1", name="bv1_sb")
            bvb = pp.tile([P, H, HD], FP32, tag="bvb", name="bvb_sb")
            maskf = pp.tile([P, P], FP32, tag="maskf", name="maskf_sb")
            maskb = pp.tile([P, P], BF16, tag="maskb", name="maskb_sb")
            qT = [pp.tile([P, S], BF16, tag=f"qT{ct}", name=f"qT{ct}_sb")
                  for ct in range(NCT)]
            kT = [pp.tile([P, S], BF16, tag=f"kT{ct}", name=f"kT{ct}_sb")
                  for ct in range(NCT)]
            vaug = [pp.tile([P, H, HD + 1], BF16, tag=f"va{rt}", name=f"va{rt}_sb")
                    for rt in range(NRT)]
            aT = [pp.tile([P, S], BF16, tag=f"aT{ct}", name=f"aT{ct}_sb")
                  for ct in range(NCT)]
            # z rows live at quadrant-aligned partitions {0,32,64,96} (the only
            # legal single-partition access bases): tile [2][i] holds head-pair
            # hp's denominator for head i at partition 32*hp.
            zq = [[pp.tile([P, QB], FP32, tag=f"zq{qb}_{i}", name=f"zq{qb}{i}_sb")
                   for i in range(2)] for qb in range(NQB)]
            zinv = [[pp.tile([P, QB], FP32, tag=f"zi{qb}_{i}", name=f"zi{qb}{i}_sb")
                     for i in range(2)] for qb in range(NQB)]

            # ---- input DMA (few big transfers: per-DMA overhead dominates;
            # hidT split in 4 so the first QKV matmuls start early) ----
            HK = KD // 4
            for h4 in range(4):
                nc.sync.dma_start(
                    hidT[:, h4 * HK:(h4 + 1) * HK, :],
                    hidT_d[h4 * HK * P:(h4 + 1) * HK * P, :]
                    .rearrange("(j p) s -> p j s", p=P))
            nc.sync.dma_start(wq[:], wq_d.rearrange("(j p) c -> p j c", p=P))
            nc.sync.dma_start(wk[:], wk_d.rearrange("(j p) c -> p j c", p=P))
            nc.sync.dma_start(wv[:], wv_d.rearrange("(j p) c -> p j c", p=P))
            nc.sync.dma_start(wp[:], wp_d.rearrange("(ct p) d -> p ct d", p=P))
            nc.sync.dma_start(bq[:], bq_d.rearrange("(ct p) -> p ct", p=P))
            nc.sync.dma_start(bk[:], bk_d.rearrange("(ct p) -> p ct", p=P))
            nc.sync.dma_start(bv1[:], bv_d[None, :])

            # broadcast bv across partitions: bvb[p, h, dd] = bv[64h+dd]
            nc.gpsimd.partition_broadcast(
                bvb.rearrange("p h d -> p (h d)"), bv1[:], channels=P)

            # triangular mask band: maskb[kk, u] = 1 if u >= kk else 0
            nc.vector.memset(maskf[:], 1.0)
            nc.gpsimd.affine_select(
                out=maskf[:], in_=maskf[:], compare_op=OP.is_ge, fill=0.0,
                base=0, channel_multiplier=-1, pattern=[[1, P]])
            nc.vector.tensor_copy(maskb[:], maskf[:])

            # ---- PE work generators ----
            def qkv_groups(rc):
                """QKV matmul groups for sequence chunk rc (list of thunks)."""
                gs = []
                cols = slice(rc * RC, (rc + 1) * RC)

                def qk_group(ct, wt, bt, dst):
                    ccols = slice(ct * P, (ct + 1) * P)
                    ps = mmp.tile([P, RC], FP32, tag="mm", name="qk_ps")
                    for j in range(KD):
                        nc.tensor.matmul(
                            ps[:], wt[:, j, ccols], hidT[:, j, cols],
                            start=(j == 0), stop=(j == KD - 1))
                    nc.vector.tensor_scalar_add(
                        dst[ct][:, cols], ps[:], bt[:, ct:ct + 1])

                def v_group(rt):
                    rows = slice(rt * P, (rt + 1) * P)
                    ps = mmp.tile([P, C], FP32, tag="mm", name="v_ps")
                    for j in range(KD):
                        nc.tensor.matmul(
                            ps[:], hidT[:, j, rows], wv[:, j, :],
                            start=(j == 0), stop=(j == KD - 1))
                    nc.vector.tensor_tensor(
                        vaug[rt][:, :, 0:HD],
                        ps.rearrange("p (h d) -> p h d", h=H), bvb[:], OP.add)
                    nc.vector.memset(vaug[rt][:, :, HD:HD + 1], 1.0)

                for ct in range(NCT):
                    for (wt, bt, dst) in ((wq, bq, qT), (wk, bk, kT)):
                        gs.append(lambda ct=ct, wt=wt, bt=bt, dst=dst:
                                  qk_group(ct, wt, bt, dst))
                for rt in range(rc * RC // P, (rc + 1) * RC // P):
                    gs.append(lambda rt=rt: v_group(rt))
                return gs

            def proj_groups(qb):
                """Output-projection groups for q block qb (list of thunks)."""
                gs = []

                def pj_group(rt):
                    rows = slice(rt * P, (rt + 1) * P)
                    ob = obp.tile([P, NPJ, 512], FP32, tag="ob", name="ob_t")
                    for nj in range(NPJ):
                        ncols = slice(nj * 512, (nj + 1) * 512)
                        ps = mmp.tile([P, 512], FP32, tag="mm", name="pj_ps")
                        for ct in range(NCT):
                            nc.tensor.matmul(
                                ps[:], aT[ct][:, rows], wp[:, ct, ncols],
                                start=(ct == 0), stop=(ct == NCT - 1))
                        nc.vector.tensor_copy(ob[:, nj, :], ps[:])
                    nc.sync.dma_start(out_d[rows, :],
                                      ob.rearrange("p a b -> p (a b)"))

                for rt in range(qb * QB // P, (qb + 1) * QB // P):
                    gs.append(lambda rt=rt: pj_group(rt))
                return gs

            # ---- attention for one q block, with PE filler interleave ----
            def attention(qb, fillers, uns):
                qcols = slice(qb * QB, (qb + 1) * QB)
                nkt = (qb + 1) * QB // P
                nsteps = NCT * nkt
                # spread filler groups roughly evenly over attention steps
                fill_at = {}
                for idx in range(len(fillers)):
                    pos = min(nsteps - 1, (idx * nsteps) // max(1, len(fillers)))
                    fill_at.setdefault(pos, []).append(fillers[idx])
                step = 0

                for hp in range(NCT):
                    pv2 = [pvp.tile([HD + 1, QB], FP32, tag="pv",
                                    name=f"pv_{qb}_{hp}_{i}") for i in range(2)]

                    def probs(j, qb=qb, hp=hp):
                        """scores (both heads) + one batched exp for k tile j."""
                        kcols = slice(j * P, (j + 1) * P)
                        o = (j - qb * QB // P) * P  # >=0 on diagonal tiles
                        qs = slice(qb * QB + max(o, 0), (qb + 1) * QB)
                        w = QB - max(o, 0)
                        sc2 = scp.tile([P, 2, QB], FP32, tag="sc",
                                       name=f"sc_{qb}_{hp}_{j}")
                        for i in range(2):
                            hrows = slice(i * HD, (i + 1) * HD)
                            nc.tensor.matmul(sc2[:, i, :w], kT[hp][hrows, kcols],
                                             qT[hp][hrows, qs],
                                             start=True, stop=True)
                        pt2 = ptp.tile([P, 2, QB], BF16, tag="pt", name="pt_t")
                        flat_sc = sc2.rearrange("p a b -> p (a b)")
                        flat_pt = pt2.rearrange("p a b -> p (a b)")
                        nc.scalar.activation(flat_pt[:, :QB + w], flat_sc[:, :QB + w],
                                             AF.Exp, scale=float(HD) ** -0.5)
                        if o >= 0:
                            for i in range(2):
                                nc.vector.tensor_tensor(
                                    pt2[:, i, 0:P - 1], pt2[:, i, 0:P - 1],
                                    maskb[:, 0:P - 1], OP.mult)
                        return pt2, w

                    # software pipeline, j in pairs: [PV(g), PV(g+1)] then
                    # [scores(g+2), scores(g+3)] so the tensor engine switches
                    # tiling mode once per pair instead of once per k tile.
                    pending = [probs(0), probs(1)]
                    for g in range(0, nkt, 2):
                        pvpair = [pending.pop(0), pending.pop(0)]
                        for jo in range(2):
                            j = g + jo
                            pt2, w = pvpair[jo]
                            for i in range(2):
                                nc.tensor.matmul(
                                    pv2[i][:, QB - w:], vaug[j][:, 2 * hp + i, :],
                                    pt2[:, i, :w],
                                    start=(j == 0), stop=(j == nkt - 1),
                                    skip_group_check=True)
                        if g + 3 < nkt:
                            pending += [probs(g + 2), probs(g + 3)]
                        for f in fill_at.pop(step, ()):
                            f()
                        for f in fill_at.pop(step + 1, ()):
                            f()
                        step += 2
                    # evacuate PV psum fast: unnormalized numerators into
                    # base-0 staging (TT inputs must share a base partition,
                    # so the normalize multiply needs them at partition 0),
                    # plus the denominator row into its quadrant slot.
                    for i in range(2):
                        un = unp.tile([HD, QB], BF16, tag="un",
                                      name=f"un_{qb}_{hp}_{i}")
                        nc.vector.tensor_copy(un[:], pv2[i][0:HD, :])
                        uns[2 * hp + i] = un
                        nc.vector.tensor_copy(
                            zq[qb][i][32 * hp:32 * hp + 1, :],
                            pv2[i][HD:HD + 1, :])
                assert not fill_at, "unplaced filler groups"

            def normalize(qb, uns):
                # partition_broadcast only works from a base-0 source (HW
                # ucode limitation, verified) -> stage each 1/z row at
                # partition 0 first.
                qcols = slice(qb * QB, (qb + 1) * QB)
                for i in range(2):
                    nc.vector.reciprocal(zinv[qb][i][:], zq[qb][i][:])
                for hp in range(NCT):
                    for i in range(2):
                        if hp == 0:
                            src = zinv[qb][i][0:1, :]
                        else:
                            src = wkp.tile([1, QB], FP32, tag="zrow",
                                           name="zrow_t")
                            nc.vector.tensor_copy(
                                src[:], zinv[qb][i][32 * hp:32 * hp + 1, :])
                        bz = bzp.tile([HD, QB], FP32, tag="bz", name="bz_t")
                        nc.gpsimd.partition_broadcast(bz[:], src[:], channels=HD)
                        nc.vector.tensor_tensor(
                            aT[hp][i * HD:(i + 1) * HD, qcols],
                            uns[2 * hp + i][:], bz[:], OP.mult)

            # ---- main schedule ----
            for g in qkv_groups(0):
                g()
            for qb in range(NQB):
                fillers = []
                if qb + 1 < NQB:
                    fillers += qkv_groups(qb + 1)
                if qb > 0:
                    fillers += proj_groups(qb - 1)
                uns = [None] * H
                attention(qb, fillers, uns)
                normalize(qb, uns)
            for g in proj_groups(NQB - 1):
                g()

    nc.compile()
    return nc


def _shard_inputs(hidden_states, w_attn, b_attn, w_proj):
    bf16 = ml_dtypes.bfloat16
    in_maps = []
    for c in range(8):
        b, m = divmod(c, 2)
        sl = slice(m * C, (m + 1) * C)
        in_maps.append(dict(
            hidT=np.ascontiguousarray(hidden_states[b].T).astype(bf16),
            wq=np.ascontiguousarray(w_attn[:, sl]).astype(bf16),
            wk=np.ascontiguousarray(w_attn[:, D + m * C:D + (m + 1) * C]).astype(bf16),
            wv=np.ascontiguousarray(w_attn[:, 2 * D + m * C:2 * D + (m + 1) * C]).astype(bf16),
            bq=np.ascontiguousarray(b_attn[sl]),
            bk=np.ascontiguousarray(b_attn[D + m * C:D + (m + 1) * C]),
            bv=np.ascontiguousarray(b_attn[2 * D + m * C:2 * D + (m + 1) * C]),
            wp=np.ascontiguousarray(w_proj[sl, :]).astype(bf16),
        ))
    return in_maps


def _assemble(outs, b_proj):
    return np.stack([outs[2 * b] + outs[2 * b + 1] + b_proj[None, :]
                     for b in range(4)]).astype(np.float32)


def kernel(hidden_states, w_attn, b_attn, w_proj, b_proj):
    hidden_states = np.asarray(hidden_states, dtype=np.float32)
    w_attn = np.asarray(w_attn, dtype=np.float32)
    b_attn = np.asarray(b_attn, dtype=np.float32)
    w_proj = np.asarray(w_proj, dtype=np.float32)
    b_proj = np.asarray(b_proj, dtype=np.float32)

    if "nc" not in _NC_CACHE:
        _NC_CACHE["nc"] = _build_kernel()
    nc = _NC_CACHE["nc"]

    in_maps = _shard_inputs(hidden_states, w_attn, b_attn, w_proj)
    res = run_bass_kernel_spmd(nc, in_maps, core_ids=list(range(8)))
    outs = [r["out"] for r in res.results]
    return _assemble(outs, b_proj)


if __name__ == "__main__":
    rng = np.random.default_rng(0)
    hs = rng.standard_normal((4, S, D)).astype(np.float32)
    wa = (rng.standard_normal((D, 3 * D)) * 0.02).astype(np.float32)
    ba = np.zeros(3 * D, np.float32)
    wpj = (rng.standard_normal((D, D)) * 0.02).astype(np.float32)
    bpj = np.zeros(D, np.float32)
    out = kernel(hs, wa, ba, wpj, bpj)
    print("kernel out", out.shape, out.dtype, float(np.abs(out).max()))
